# revision 1
# baseline (speedup 1.0000x reference)
"""Trainium2 Bass kernel for nn_BoundaryKDV4 (boundary-KL distillation loss).

Contract: kernel(**inputs) takes FULL inputs (preds_S, preds_T, outputs_T:
[2,14,96,96,96] f32), shards across 8 NeuronCores internally, and returns the
FULL output (scalar f32 loss), matching reference.py semantics.

Sharding: core = (b, hq) with b in {0,1}, hq in {0..3}; each core handles 24
H-slices of one batch. outputs_T shards carry a 1-slice halo on each side and
an extra "mask channel" (index 14) set to -2e38 on valid slices / +2e38 on
out-of-range halo slices, so the argmax one-hot of padding slices is
identically zero with a core-uniform SPMD program. Host supplies the oT shard
pre-transposed to (slice, w, chan, d) so each DMA is one contiguous run per
partition.

Per-core pipeline:
  Phase A (KL): flat [128, 14, j] layout. expT/expS via ACT (bf16), d=sT-sS,
    q=expT*d, class sums via bf16 add trees into persistent [128,1728] tiles;
    then one batched voxel pass: pk = sumq/sumT - ln sumT + ln sumS (2 ACT
    table-set switches total). pk -> DRAM scratch, reloaded as [96W, (h,d)].
  Phase B/C (boundary): [96W, (c,h,d)] layout. Tree-max over 15 channels +
    broadcast is_ge -> one-hot (bf16). H-box = 2 aligned bf16 shifted adds.
    W-box = tridiagonal band matmul on PE; D-box = 3 PSUM-accumulated
    d-shifted matmuls. ind = [0<box<27] via Square(box-13.5) < 169.
    n[k] = sum(ind), num[k] = sum(ind*pk) accumulated on PE with ones-column
    matmuls into two persistent PSUM banks, columns folded as (d%32, c).
Host combines the 8 partial (n, num) pairs and applies the final
KLDivLoss(mean)-style normalization.
"""

import numpy as np

B, C, H, W, D = 2, 14, 96, 96, 96
CM = C + 1          # channels incl. mask
HQ = H // 4         # 24 h-slices per core
S = HQ + 2          # 26 oT slices incl. halo
WD = W * D          # 9216
V = HQ * WD         # 221184 voxels per core
NCORES = 8
BIG = 2.0e38        # +/- mask channel values (finite, beats any data)

# phase A chunking: V = 128 partitions * 1728; 1728 = A_CHUNKS * AJ
AJ = 288
A_CHUNKS = 6
VP = V // 128       # 1728 per partition

_CACHE = {}


def _build_program():
    import concourse.bacc as bacc
    import concourse.mybir as mybir
    from concourse.mybir import AluOpType as alu
    from concourse.mybir import ActivationFunctionType as actf
    from concourse.tile import TileContext
    from contextlib import ExitStack
    import ml_dtypes

    f32 = mybir.dt.float32
    bf16 = mybir.dt.bfloat16
    bfnp = ml_dtypes.bfloat16

    nc = bacc.Bacc("TRN2", target_bir_lowering=False)

    # oT shard pre-transposed by host to (s, w, c, d)
    ot = nc.dram_tensor("ot15", [S, W, CM, D], f32, kind="ExternalInput")
    ps = nc.dram_tensor("ps", [C, V], f32, kind="ExternalInput")
    pt = nc.dram_tensor("pt", [C, V], f32, kind="ExternalInput")
    nn_out = nc.dram_tensor("nn_out", [2, C], f32, kind="ExternalOutput")

    band_np = np.zeros((W, W), dtype=bfnp)
    for i in range(W):
        for j in range(max(0, i - 1), min(W, i + 2)):
            band_np[i, j] = 1.0
    band_h = nc.inline_tensor(band_np, name="bandw")
    ones_h = nc.inline_tensor(np.ones((W, 1), dtype=bfnp), name="onesw")

    with TileContext(nc) as tc, ExitStack() as es:
        # ---------------- constants ----------------
        cpool = es.enter_context(tc.tile_pool(name="consts", bufs=1))
        band_t = cpool.tile([W, W], bf16, name="band_t")
        ones_t = cpool.tile([W, 1], bf16, name="ones_t")
        bias_t = cpool.tile([W, 1], f32, name="bias_t")
        nc.sync.dma_start(band_t[:], band_h[:])
        nc.sync.dma_start(ones_t[:], ones_h[:])
        nc.vector.memset(bias_t[:], -13.5)

        dram_pool = es.enter_context(tc.tile_pool(name="dramp", bufs=1, space="DRAM"))
        pk_dram = dram_pool.tile([V], bf16, name="pk_dram")

        # ---------------- phase A: per-voxel KL (pk) ----------------
        ps_r = ps[:].rearrange("c (p a j) -> a p c j", p=128, a=A_CHUNKS, j=AJ)
        pt_r = pt[:].rearrange("c (p a j) -> a p c j", p=128, a=A_CHUNKS, j=AJ)

        apool = es.enter_context(tc.tile_pool(name="asums", bufs=1))
        sumT = apool.tile([128, A_CHUNKS, AJ], bf16, name="sumT")
        sumS = apool.tile([128, A_CHUNKS, AJ], bf16, name="sumS")
        sumq = apool.tile([128, A_CHUNKS, AJ], bf16, name="sumq")

        with tc.tile_pool(name="aload", bufs=2) as alp, \
             tc.tile_pool(name="awork", bufs=2) as awp:
            for a in range(A_CHUNKS):
                sS_t = alp.tile([128, C, AJ], f32, name="sS_t", tag="sS")
                sT_t = alp.tile([128, C, AJ], f32, name="sT_t", tag="sT")
                nc.sync.dma_start(sS_t[:], ps_r[a])
                nc.sync.dma_start(sT_t[:], pt_r[a])

                expT = awp.tile([128, C, AJ], bf16, name="expT", tag="expT")
                expS = awp.tile([128, C, AJ], bf16, name="expS", tag="expS")
                d_t = awp.tile([128, C, AJ], bf16, name="d_t", tag="d")
                q_t = awp.tile([128, C, AJ], bf16, name="q_t", tag="q")
                nc.scalar.activation(expT[:], sT_t[:], actf.Exp)
                nc.scalar.activation(expS[:], sS_t[:], actf.Exp)
                nc.vector.tensor_tensor(d_t[:], sT_t[:], sS_t[:], alu.subtract)
                nc.vector.tensor_tensor(q_t[:], expT[:], d_t[:], alu.mult)

                for src, sm_all in ((expT, sumT), (expS, sumS), (q_t, sumq)):
                    nm = sm_all.name[:4]
                    w7 = awp.tile([128, 7, AJ], bf16, name=f"w7{nm}", tag=f"w7{nm}")
                    w3 = awp.tile([128, 3, AJ], bf16, name=f"w3{nm}", tag=f"w3{nm}")
                    sm = sm_all[:, a, :]
                    nc.vector.tensor_tensor(w7[:], src[:, 0:7, :], src[:, 7:14, :], alu.add)
                    nc.vector.tensor_tensor(w3[:], w7[:, 0:3, :], w7[:, 3:6, :], alu.add)
                    nc.vector.tensor_tensor(sm, w3[:, 0, :], w3[:, 1, :], alu.add)
                    nc.vector.tensor_tensor(sm, sm, w3[:, 2, :], alu.add)
                    nc.vector.tensor_tensor(sm, sm, w7[:, 6, :], alu.add)

        # batched voxel stage: 2 table-set switches total
        with tc.tile_pool(name="avox", bufs=1) as avp:
            lnT = avp.tile([128, VP], bf16, name="lnT")
            lnS = avp.tile([128, VP], bf16, name="lnS")
            inv = avp.tile([128, VP], bf16, name="inv")
            pk_c = avp.tile([128, VP], bf16, name="pk_c")
            sT_f = sumT[:].rearrange("p a j -> p (a j)")
            sS_f = sumS[:].rearrange("p a j -> p (a j)")
            sq_f = sumq[:].rearrange("p a j -> p (a j)")
            nc.scalar.activation(lnT[:], sT_f, actf.Ln)
            nc.scalar.activation(lnS[:], sS_f, actf.Ln)
            # inv = exp(-lnT) = 1/sumT
            nc.scalar.activation(inv[:], lnT[:], actf.Exp, scale=-1.0)
            nc.vector.tensor_tensor(pk_c[:], sq_f, inv[:], alu.mult)
            nc.vector.tensor_tensor(pk_c[:], pk_c[:], lnT[:], alu.subtract)
            nc.vector.tensor_tensor(pk_c[:], pk_c[:], lnS[:], alu.add)
            nc.sync.dma_start(
                pk_dram[:].rearrange("(p j) -> p j", p=128), pk_c[:]
            )

        # ---------------- phase B/C: boundary + reductions ----------------
        # pk in [96W, (h, d)] layout
        bpool = es.enter_context(tc.tile_pool(name="bconst", bufs=1))
        pk_w = bpool.tile([W, HQ, D], bf16, name="pk_w")
        nc.sync.dma_start(
            pk_w[:], pk_dram[:].rearrange("(h w d) -> w h d", h=HQ, w=W, d=D)
        )

        # one-hot of all 26 slices, flat (no d padding needed)
        oh = bpool.tile([W, S, C, D], bf16, name="oh")

        psum_acc = es.enter_context(tc.tile_pool(name="psacc", bufs=1, space="PSUM"))
        nacc = psum_acc.tile([1, 448], f32, name="nacc")
        numacc = psum_acc.tile([1, 448], f32, name="numacc")

        with tc.tile_pool(name="otload", bufs=2) as otp, \
             tc.tile_pool(name="mwork", bufs=2) as mp, \
             tc.tile_pool(name="treework", bufs=1) as twp, \
             tc.tile_pool(name="hbwork", bufs=1) as hbp, \
             tc.tile_pool(name="gwork", bufs=2) as gp, \
             tc.tile_pool(name="npwork", bufs=1) as npp, \
             tc.tile_pool(name="boxps", bufs=6, space="PSUM") as bxp:

            # c-chunks for the band matmuls (PSUM bank <= 512 f32 cols)
            CCH = [(0, 5), (5, 10), (10, 14)]

            def emit_tree_cmp(s0, ns):
                """load oT slices [s0, s0+ns), tree-max over 15 ch, cmp -> oh"""
                oT_t = otp.tile([W, CM, 2, D], f32, name="oT_t", tag="oT")
                for i in range(ns):
                    nc.sync.dma_start(
                        oT_t[:, :, i, :], ot_r := ot[s0 + i]
                    )
                m_t = mp.tile([W, 2, D], f32, name="m_t", tag="m")
                w7 = twp.tile([W, 7, 2, D], f32, name="mw7", tag="mw7")
                w3 = twp.tile([W, 3, 2, D], f32, name="mw3", tag="mw3")
                x = oT_t
                nc.vector.tensor_tensor(
                    w7[:, :, 0:ns, :], x[:, 0:7, 0:ns, :], x[:, 7:14, 0:ns, :], alu.max
                )
                nc.vector.tensor_tensor(
                    w3[:, :, 0:ns, :], w7[:, 0:3, 0:ns, :], w7[:, 3:6, 0:ns, :], alu.max
                )
                nc.vector.tensor_tensor(
                    m_t[:, 0:ns, :], w3[:, 0, 0:ns, :], w3[:, 1, 0:ns, :], alu.max
                )
                nc.vector.tensor_tensor(
                    m_t[:, 0:ns, :], m_t[:, 0:ns, :], w3[:, 2, 0:ns, :], alu.max
                )
                nc.vector.tensor_tensor(
                    m_t[:, 0:ns, :], m_t[:, 0:ns, :], w7[:, 6, 0:ns, :], alu.max
                )
                nc.vector.tensor_tensor(
                    m_t[:, 0:ns, :], m_t[:, 0:ns, :], x[:, 14, 0:ns, :], alu.max
                )
                # one-hot: broadcast compare over the class dim
                nc.vector.tensor_tensor(
                    oh[:, s0 : s0 + ns, :, :],
                    oT_t[:, 0:14, 0:ns, :].rearrange("w c s d -> w s c d"),
                    m_t[:, 0:ns, None, :].broadcast_to([W, ns, C, D]),
                    alu.is_ge,
                )

            def emit_group(g0, ng, last_group):
                """boundary + accumulation for own slices g in [g0, g0+ng)"""
                # hb[j] = oh[g0+j] + oh[g0+j+1] + oh[g0+j+2] (d padded to 100)
                hb_t = hbp.tile([W, 4, C, 100], bf16, name="hb_t", tag="hb")
                nc.vector.memset(hb_t[:, :, :, 0:2], 0.0)
                nc.vector.memset(hb_t[:, :, :, 98:100], 0.0)
                nc.vector.tensor_tensor(
                    hb_t[:, 0:ng, :, 2 : 2 + D],
                    oh[:, g0 : g0 + ng, :, :],
                    oh[:, g0 + 2 : g0 + 2 + ng, :, :],
                    alu.add,
                )
                nc.vector.tensor_tensor(
                    hb_t[:, 0:ng, :, 2 : 2 + D],
                    hb_t[:, 0:ng, :, 2 : 2 + D],
                    oh[:, g0 + 1 : g0 + 1 + ng, :, :],
                    alu.add,
                )

                u_t = gp.tile([W, 4, C, D], bf16, name="u_t", tag="u")
                ind_t = gp.tile([W, 4, C, D], bf16, name="ind_t", tag="ind")
                np_t = npp.tile([W, 4, C, D], bf16, name="np_t", tag="np")

                for gi in range(ng):
                    for (c0, c1) in CCH:
                        cl = c1 - c0
                        box_ps = bxp.tile([W, 5 * D], f32, name="box_ps", tag="box")
                        box_v = box_ps[:].rearrange("w (c d) -> w c d", c=5)[
                            :, 0:cl, :
                        ]
                        for dd in range(3):
                            nc.tensor.matmul(
                                box_v,
                                band_t[:],
                                hb_t[:, gi, c0:c1, 1 + dd : 1 + dd + D],
                                start=(dd == 0),
                                stop=(dd == 2),
                            )
                        # u = (box - 13.5)^2 ; boundary iff u < 169 (0<box<27)
                        nc.scalar.activation(
                            u_t[:, gi, c0:c1, :],
                            box_v,
                            actf.Square,
                            bias=bias_t[:],
                        )
                nc.vector.tensor_scalar(
                    ind_t[:, 0:ng, :, :], u_t[:, 0:ng, :, :], 169.0, None, alu.is_lt
                )
                nc.vector.tensor_tensor(
                    np_t[:, 0:ng, :, :],
                    ind_t[:, 0:ng, :, :],
                    pk_w[:, g0 : g0 + ng, None, :].broadcast_to([W, ng, C, D]),
                    alu.mult,
                )
                # colsum matmuls: accumulate per (type, g, d-chunk) into psum,
                # cols laid out (d % 32, c) so different d-chunks fold together
                for gi in range(ng):
                    for ti, (src, accp) in enumerate(((ind_t, nacc), (np_t, numacc))):
                        for dc in range(3):
                            rhs = src[:, gi, :, 32 * dc : 32 * (dc + 1)].rearrange(
                                "w c d -> w d c"
                            )
                            is_first = first_cs[ti]
                            first_cs[ti] = False
                            is_last = last_group and gi == ng - 1 and dc == 2
                            nc.tensor.matmul(
                                accp[:].rearrange("p (d c) -> p d c", c=C),
                                ones_t[:],
                                rhs,
                                start=is_first,
                                stop=is_last,
                                skip_group_check=True,
                            )

            first_cs = [True, True]  # per type (n, num): first colsum matmul?

            # 13 loads of 2 slices each cover s in [0, 26); own-g group k
            # (g in [4k, 4k+4)) needs oh slices s <= 4k+5, ready after load
            # ld >= 2k+2.
            next_k = 0
            for ld in range(13):
                emit_tree_cmp(2 * ld, 2)
                while next_k < 6 and 2 * next_k + 2 <= ld:
                    emit_group(4 * next_k, 4, next_k == 5)
                    next_k += 1
            while next_k < 6:
                emit_group(4 * next_k, 4, next_k == 5)
                next_k += 1

            # final: reduce (d%32) out of the accumulators, write [2, C]
            res_t = mp.tile([1, 2, C], f32, name="res_t", tag="res")
            nc.vector.tensor_reduce(
                res_t[:, 0, :],
                nacc[:].rearrange("p (d c) -> p c d", c=C),
                mybir.AxisListType.X,
                alu.add,
            )
            nc.vector.tensor_reduce(
                res_t[:, 1, :],
                numacc[:].rearrange("p (d c) -> p c d", c=C),
                mybir.AxisListType.X,
                alu.add,
            )
            nc.sync.dma_start(
                nn_out[:].rearrange("a c -> (a c)")[None, :],
                res_t[:].rearrange("p a c -> p (a c)"),
            )

    nc.compile()
    return nc


def _get_program():
    if "nc" not in _CACHE:
        _CACHE["nc"] = _build_program()
    return _CACHE["nc"]


def _make_in_maps(preds_S, preds_T, outputs_T):
    in_maps = []
    for core in range(NCORES):
        b, hq = divmod(core, 4)
        h0 = HQ * hq
        ot15 = np.empty((CM, S, W, D), dtype=np.float32)
        lo, hi = h0 - 1, h0 + HQ + 1
        slo, shi = max(0, lo), min(H, hi)
        ot15[:C, slo - lo : shi - lo] = outputs_T[b, :, slo:shi]
        ot15[C, :] = -BIG
        if lo < 0:
            ot15[:C, 0] = 0.0
            ot15[C, 0] = BIG
        if hi > H:
            ot15[:C, S - 1] = 0.0
            ot15[C, S - 1] = BIG
        in_maps.append(
            {
                # (c, s, w, d) -> (s, w, c, d): one contiguous run/partition
                "ot15": np.ascontiguousarray(ot15.transpose(1, 2, 0, 3)),
                "ps": np.ascontiguousarray(
                    preds_S[b, :, h0 : h0 + HQ].reshape(C, V)
                ),
                "pt": np.ascontiguousarray(
                    preds_T[b, :, h0 : h0 + HQ].reshape(C, V)
                ),
            }
        )
    return in_maps


def _combine(results):
    n = np.zeros((B, C), dtype=np.float64)
    num = np.zeros((B, C), dtype=np.float64)
    for core, res in enumerate(results):
        b = core // 4
        nn = np.asarray(res["nn_out"], dtype=np.float64)
        n[b] += nn[0]
        num[b] += nn[1]
    term = np.where(n > 0, num / (C * np.maximum(n, 1.0)), 0.0)
    return np.float32(term.sum())


def kernel(preds_S, preds_T, outputs_T):
    from concourse.bass_utils import run_bass_kernel_spmd

    nc = _get_program()
    in_maps = _make_in_maps(preds_S, preds_T, outputs_T)
    res = run_bass_kernel_spmd(nc, in_maps, core_ids=list(range(NCORES)))
    return np.asarray(_combine(res.results))



# revision 5
# speedup vs baseline: 1.5409x; 1.5409x over previous
"""Trainium2 Bass kernel for nn_BoundaryKDV4 (boundary-KL distillation loss).

Contract: kernel(**inputs) takes FULL inputs (preds_S, preds_T, outputs_T:
[2,14,96,96,96] f32), shards across 8 NeuronCores internally, and returns the
FULL output (scalar f32 loss), matching reference.py semantics.

Sharding: core = (b, hq) with b in {0,1}, hq in {0..3}; each core handles 24
H-slices of one batch. outputs_T shards carry a 1-slice halo on each side and
an extra "mask channel" (index 14) set to -2e38 on valid slices / +2e38 on
out-of-range halo slices, so the argmax one-hot of padding slices is
identically zero with a core-uniform SPMD program. All inputs are cast to
bf16 on the host (halves HBM traffic, doubles DVE throughput; the loss is an
average over ~1.8M voxels so the rounding noise washes out far below the
2e-2 gate). oT shards are pre-transposed to (slice, w, chan, d) so each DMA
is one contiguous run per partition.

Per-core pipeline (phase A and the oT/one-hot work interleaved so DMA, ACT,
DVE and PE all overlap):
  Phase A (KL): flat [128, 14, j] layout. expT/expS via ACT (bf16), d=sT-sS,
    q=expT*d, class sums via bf16 add trees into persistent [128,1728] tiles;
    then one batched voxel pass: pk = sumq/sumT - ln sumT + ln sumS (2 ACT
    table-set switches total). pk -> DRAM scratch, reloaded as [96W, (h,d)].
  One-hot (interleaved with A): [96W, (s, c, d)] layout. Tree-max over 15
    channels + broadcast is_ge -> one-hot (bf16) for all 26 slices.
  Phase B (boundary): H-box = 2 aligned bf16 shifted adds into a 128-row
    zero-padded tile. W-box+D-box = 3 PSUM-accumulated band matmuls on PE
    with a 128x128 zero-padded tridiagonal weight (128 weights => FWL).
    ind = [0<box<27] via Square(box-13.5) < 169 (ACT + 4x tensor_scalar).
    n[k] = sum(ind), num[k] = sum(ind*pk) accumulated on PE with ones-column
    matmuls into two persistent PSUM banks, columns folded as (c, d%32) so
    the moving operand streams contiguously.
Host combines the 8 partial (n, num) pairs and applies the final
KLDivLoss(mean)-style normalization.
"""

import numpy as np

B, C, H, W, D = 2, 14, 96, 96, 96
CM = C + 1          # channels incl. mask
HQ = H // 4         # 24 h-slices per core
S = HQ + 2          # 26 oT slices incl. halo
WD = W * D          # 9216
V = HQ * WD         # 221184 voxels per core
NCORES = 8
BIG = 2.0e38        # +/- mask channel values (finite, beats any data)
WP = 128            # padded partition count for the band matmuls

# phase A chunking: V = 128 partitions * 1728; 1728 = A_CHUNKS * AJ
AJ = 288
A_CHUNKS = 6
VP = V // 128       # 1728 per partition

_CACHE = {}


def _build_program():
    import concourse.bacc as bacc
    import concourse.mybir as mybir
    from concourse.mybir import AluOpType as alu
    from concourse.mybir import ActivationFunctionType as actf
    from concourse.tile import TileContext
    from contextlib import ExitStack
    import ml_dtypes

    f32 = mybir.dt.float32
    bf16 = mybir.dt.bfloat16
    bfnp = ml_dtypes.bfloat16

    nc = bacc.Bacc("TRN2", target_bir_lowering=False)

    # oT shard pre-transposed by host to (s, w, c, d); everything bf16
    ot = nc.dram_tensor("ot15", [S, W, CM, D], bf16, kind="ExternalInput")
    ps = nc.dram_tensor("ps", [C, V], bf16, kind="ExternalInput")
    pt = nc.dram_tensor("pt", [C, V], bf16, kind="ExternalInput")
    nn_out = nc.dram_tensor("nn_out", [2, C], f32, kind="ExternalOutput")

    # 128x128 zero-padded tridiagonal band: 128 weight columns enable FWL
    band_np = np.zeros((WP, WP), dtype=bfnp)
    for i in range(W):
        for j in range(max(0, i - 1), min(W, i + 2)):
            band_np[i, j] = 1.0
    band_h = nc.inline_tensor(band_np, name="bandw")
    ones_h = nc.inline_tensor(np.ones((W, 1), dtype=bfnp), name="onesw")

    with TileContext(nc) as tc, ExitStack() as es:
        # ---------------- constants ----------------
        cpool = es.enter_context(tc.tile_pool(name="consts", bufs=1))
        band_t = cpool.tile([WP, WP], bf16, name="band_t")
        ones_t = cpool.tile([W, 1], bf16, name="ones_t")
        bias_t = cpool.tile([W, 1], f32, name="bias_t")
        nc.sync.dma_start(band_t[:], band_h[:])
        nc.sync.dma_start(ones_t[:], ones_h[:])
        nc.vector.memset(bias_t[:], -13.5)

        dram_pool = es.enter_context(tc.tile_pool(name="dramp", bufs=1, space="DRAM"))
        pk_dram = dram_pool.tile([V], bf16, name="pk_dram")

        # persistent one-hot for all 26 slices
        bpool = es.enter_context(tc.tile_pool(name="bconst", bufs=1))
        oh = bpool.tile([W, S, C, D], bf16, name="oh")
        pk_w = bpool.tile([W, HQ, D], bf16, name="pk_w")
        # two alternating 128-row H-box buffers (rows 96.. and d-pad cols
        # zeroed once; the band matmul streams them as the moving operand)
        hb_ts = [bpool.tile([WP, 4, C, 100], bf16, name=f"hb{i}") for i in range(2)]
        for hb in hb_ts:
            nc.vector.memset(hb[W:WP, :, :, :], 0.0)
            nc.vector.memset(hb[0:W, :, :, 0:2], 0.0)
            nc.vector.memset(hb[0:W, :, :, 98:100], 0.0)

        psum_acc = es.enter_context(tc.tile_pool(name="psacc", bufs=1, space="PSUM"))
        nacc = psum_acc.tile([1, 448], f32, name="nacc")
        numacc = psum_acc.tile([1, 448], f32, name="numacc")

        # ---------------- phase A tiles ----------------
        ps_r = ps[:].rearrange("c (p a j) -> a p c j", p=128, a=A_CHUNKS, j=AJ)
        pt_r = pt[:].rearrange("c (p a j) -> a p c j", p=128, a=A_CHUNKS, j=AJ)

        apool = es.enter_context(tc.tile_pool(name="asums", bufs=1))
        sumT = apool.tile([128, A_CHUNKS, AJ], bf16, name="sumT")
        sumS = apool.tile([128, A_CHUNKS, AJ], bf16, name="sumS")
        sumq = apool.tile([128, A_CHUNKS, AJ], bf16, name="sumq")

        mp = es.enter_context(tc.tile_pool(name="mwork", bufs=2))
        ap_es = ExitStack()
        alp = ap_es.enter_context(tc.tile_pool(name="aload", bufs=2))
        awp = ap_es.enter_context(tc.tile_pool(name="awork", bufs=2))
        otp = ap_es.enter_context(tc.tile_pool(name="otload", bufs=2))
        twp = ap_es.enter_context(tc.tile_pool(name="treework", bufs=2))

        def emit_a_chunk(a):
            sS_t = alp.tile([128, C, AJ], bf16, name="sS_t", tag="sS")
            sT_t = alp.tile([128, C, AJ], bf16, name="sT_t", tag="sT")
            nc.sync.dma_start(sS_t[:], ps_r[a])
            nc.sync.dma_start(sT_t[:], pt_r[a])

            expT = awp.tile([128, C, AJ], bf16, name="expT", tag="expT")
            expS = awp.tile([128, C, AJ], bf16, name="expS", tag="expS")
            q_t = awp.tile([128, C, AJ], bf16, name="q_t", tag="q")
            nc.scalar.activation(expT[:], sT_t[:], actf.Exp)
            nc.scalar.activation(expS[:], sS_t[:], actf.Exp)
            # q = expT * (sT - sS), with the difference written in place
            nc.vector.tensor_tensor(q_t[:], sT_t[:], sS_t[:], alu.subtract)
            nc.vector.tensor_tensor(q_t[:], expT[:], q_t[:], alu.mult)

            # class-sum trees, in place (expT/expS are dead after q)
            for src, sm_all in ((expT, sumT), (expS, sumS), (q_t, sumq)):
                sm = sm_all[:, a, :]
                nc.vector.tensor_tensor(
                    src[:, 0:7, :], src[:, 0:7, :], src[:, 7:14, :], alu.add
                )
                nc.vector.tensor_tensor(
                    src[:, 0:3, :], src[:, 0:3, :], src[:, 3:6, :], alu.add
                )
                nc.vector.tensor_tensor(sm, src[:, 0, :], src[:, 1, :], alu.add)
                nc.vector.tensor_tensor(sm, sm, src[:, 2, :], alu.add)
                nc.vector.tensor_tensor(sm, sm, src[:, 6, :], alu.add)

        def emit_tree_cmp(s0, ns):
            """load oT slices [s0, s0+ns), tree-max over 15 ch, cmp -> oh"""
            oT_t = otp.tile([W, CM, 2, D], bf16, name="oT_t", tag="oT")
            for i in range(ns):
                nc.sync.dma_start(oT_t[:, :, i, :], ot[s0 + i])
            m_t = mp.tile([W, 2, D], bf16, name="m_t", tag="m")
            w7 = twp.tile([W, 7, 2, D], bf16, name="mw7", tag="mw7")
            w3 = twp.tile([W, 3, 2, D], bf16, name="mw3", tag="mw3")
            x = oT_t
            nc.vector.tensor_tensor(
                w7[:, :, 0:ns, :], x[:, 0:7, 0:ns, :], x[:, 7:14, 0:ns, :], alu.max
            )
            nc.vector.tensor_tensor(
                w3[:, :, 0:ns, :], w7[:, 0:3, 0:ns, :], w7[:, 3:6, 0:ns, :], alu.max
            )
            nc.vector.tensor_tensor(
                m_t[:, 0:ns, :], w3[:, 0, 0:ns, :], w3[:, 1, 0:ns, :], alu.max
            )
            nc.vector.tensor_tensor(
                m_t[:, 0:ns, :], m_t[:, 0:ns, :], w3[:, 2, 0:ns, :], alu.max
            )
            nc.vector.tensor_tensor(
                m_t[:, 0:ns, :], m_t[:, 0:ns, :], w7[:, 6, 0:ns, :], alu.max
            )
            nc.vector.tensor_tensor(
                m_t[:, 0:ns, :], m_t[:, 0:ns, :], x[:, 14, 0:ns, :], alu.max
            )
            # one-hot: broadcast compare over the class dim
            nc.vector.tensor_tensor(
                oh[:, s0 : s0 + ns, :, :],
                oT_t[:, 0:14, 0:ns, :].rearrange("w c s d -> w s c d"),
                m_t[:, 0:ns, None, :].broadcast_to([W, ns, C, D]),
                alu.is_ge,
            )

        # ---- interleaved emission: A-chunks + oT loads/one-hot ----
        # 13 tree_cmp loads of 2 slices each cover s in [0, 26)
        tc_sched = {0: [0, 1, 2], 1: [3, 4], 2: [5, 6], 3: [7, 8], 4: [9, 10],
                    5: [11, 12]}
        for a in range(A_CHUNKS):
            emit_a_chunk(a)
            for ld in tc_sched[a]:
                emit_tree_cmp(2 * ld, 2)
        ap_es.close()  # release phase-A pools before phase B allocates

        # batched voxel stage: 2 table-set switches total
        with tc.tile_pool(name="avox", bufs=1) as avp:
            lnT = avp.tile([128, VP], bf16, name="lnT")
            lnS = avp.tile([128, VP], bf16, name="lnS")
            inv = avp.tile([128, VP], bf16, name="inv")
            pk_c = avp.tile([128, VP], bf16, name="pk_c")
            sT_f = sumT[:].rearrange("p a j -> p (a j)")
            sS_f = sumS[:].rearrange("p a j -> p (a j)")
            sq_f = sumq[:].rearrange("p a j -> p (a j)")
            nc.scalar.activation(lnT[:], sT_f, actf.Ln)
            nc.scalar.activation(lnS[:], sS_f, actf.Ln)
            # inv = exp(-lnT) = 1/sumT
            nc.scalar.activation(inv[:], lnT[:], actf.Exp, scale=-1.0)
            nc.vector.tensor_tensor(pk_c[:], sq_f, inv[:], alu.mult)
            nc.vector.tensor_tensor(pk_c[:], pk_c[:], lnT[:], alu.subtract)
            nc.vector.tensor_tensor(pk_c[:], pk_c[:], lnS[:], alu.add)
            nc.sync.dma_start(
                pk_dram[:].rearrange("(p j) -> p j", p=128), pk_c[:]
            )

        # pk in [96W, (h, d)] layout
        nc.sync.dma_start(
            pk_w[:], pk_dram[:].rearrange("(h w d) -> w h d", h=HQ, w=W, d=D)
        )

        # ---------------- phase B: boundary + reductions ----------------
        with tc.tile_pool(name="gwork", bufs=2) as gp, \
             tc.tile_pool(name="npwork", bufs=2) as npp, \
             tc.tile_pool(name="boxps", bufs=6, space="PSUM") as bxp:

            # c-chunks for the band matmuls (PSUM bank <= 512 f32 cols)
            CCH = [(0, 5), (5, 10), (10, 14)]
            first_cs = [True, True]  # per type (n, num): first colsum matmul?

            def emit_group(g0, ng, last_group):
                """boundary + accumulation for own slices g in [g0, g0+ng)"""
                # hb[j] = oh[g0+j] + oh[g0+j+1] + oh[g0+j+2] (d padded to 100)
                hb_t = hb_ts[(g0 // 4) % 2]
                nc.vector.tensor_tensor(
                    hb_t[0:W, 0:ng, :, 2 : 2 + D],
                    oh[:, g0 : g0 + ng, :, :],
                    oh[:, g0 + 2 : g0 + 2 + ng, :, :],
                    alu.add,
                )
                nc.vector.tensor_tensor(
                    hb_t[0:W, 0:ng, :, 2 : 2 + D],
                    hb_t[0:W, 0:ng, :, 2 : 2 + D],
                    oh[:, g0 + 1 : g0 + 1 + ng, :, :],
                    alu.add,
                )

                u_t = gp.tile([W, 4, C, D], bf16, name="u_t", tag="u")
                ind_t = gp.tile([W, 4, C, D], bf16, name="ind_t", tag="ind")
                np_t = npp.tile([W, 4, C, D], bf16, name="np_t", tag="np")

                for gi in range(ng):
                    for (c0, c1) in CCH:
                        cl = c1 - c0
                        box_ps = bxp.tile([WP, 5 * D], f32, name="box_ps", tag="box")
                        box_v = box_ps[:].rearrange("w (c d) -> w c d", c=5)
                        for dd in range(3):
                            nc.tensor.matmul(
                                box_v[:, 0:cl, :],
                                band_t[:],
                                hb_t[:, gi, c0:c1, 1 + dd : 1 + dd + D],
                                start=(dd == 0),
                                stop=(dd == 2),
                            )
                        # u = (box - 13.5)^2 ; boundary iff u < 169 (0<box<27)
                        nc.scalar.activation(
                            u_t[:, gi, c0:c1, :],
                            box_v[0:W, 0:cl, :],
                            actf.Square,
                            bias=bias_t[:],
                        )
                nc.vector.tensor_scalar(
                    ind_t[:, 0:ng, :, :], u_t[:, 0:ng, :, :], 169.0, None, alu.is_lt
                )
                nc.vector.tensor_tensor(
                    np_t[:, 0:ng, :, :],
                    ind_t[:, 0:ng, :, :],
                    pk_w[:, g0 : g0 + ng, None, :].broadcast_to([W, ng, C, D]),
                    alu.mult,
                )
                # colsum matmuls: accumulate per (type, g, d-chunk) into psum,
                # cols laid out (c, d % 32) so the rhs streams contiguously
                for gi in range(ng):
                    for ti, (src, accp) in enumerate(((ind_t, nacc), (np_t, numacc))):
                        for dc in range(3):
                            rhs = src[:, gi, :, 32 * dc : 32 * (dc + 1)]
                            is_first = first_cs[ti]
                            first_cs[ti] = False
                            is_last = last_group and gi == ng - 1 and dc == 2
                            nc.tensor.matmul(
                                accp[:].rearrange("p (c d) -> p c d", c=C),
                                ones_t[:],
                                rhs,
                                start=is_first,
                                stop=is_last,
                                skip_group_check=True,
                            )

            for k in range(6):
                emit_group(4 * k, 4, k == 5)

            # final: reduce (d%32) out of the accumulators, write [2, C]
            res_t = mp.tile([1, 2, C], f32, name="res_t", tag="res")
            nc.vector.tensor_reduce(
                res_t[:, 0, :],
                nacc[:].rearrange("p (c d) -> p c d", c=C),
                mybir.AxisListType.X,
                alu.add,
            )
            nc.vector.tensor_reduce(
                res_t[:, 1, :],
                numacc[:].rearrange("p (c d) -> p c d", c=C),
                mybir.AxisListType.X,
                alu.add,
            )
            nc.sync.dma_start(
                nn_out[:].rearrange("a c -> (a c)")[None, :],
                res_t[:].rearrange("p a c -> p (a c)"),
            )

    nc.compile()
    return nc


def _get_program():
    if "nc" not in _CACHE:
        _CACHE["nc"] = _build_program()
    return _CACHE["nc"]


def _make_in_maps(preds_S, preds_T, outputs_T):
    import ml_dtypes

    bfnp = ml_dtypes.bfloat16
    preds_S = np.asarray(preds_S)
    preds_T = np.asarray(preds_T)
    outputs_T = np.asarray(outputs_T)
    in_maps = []
    for core in range(NCORES):
        b, hq = divmod(core, 4)
        h0 = HQ * hq
        ot15 = np.empty((CM, S, W, D), dtype=np.float32)
        lo, hi = h0 - 1, h0 + HQ + 1
        slo, shi = max(0, lo), min(H, hi)
        ot15[:C, slo - lo : shi - lo] = outputs_T[b, :, slo:shi]
        ot15[C, :] = -BIG
        if lo < 0:
            ot15[:C, 0] = 0.0
            ot15[C, 0] = BIG
        if hi > H:
            ot15[:C, S - 1] = 0.0
            ot15[C, S - 1] = BIG
        in_maps.append(
            {
                # (c, s, w, d) -> (s, w, c, d): one contiguous run/partition
                "ot15": np.ascontiguousarray(
                    ot15.transpose(1, 2, 0, 3)
                ).astype(bfnp),
                "ps": np.ascontiguousarray(
                    preds_S[b, :, h0 : h0 + HQ].reshape(C, V)
                ).astype(bfnp),
                "pt": np.ascontiguousarray(
                    preds_T[b, :, h0 : h0 + HQ].reshape(C, V)
                ).astype(bfnp),
            }
        )
    return in_maps


def _combine(results):
    n = np.zeros((B, C), dtype=np.float64)
    num = np.zeros((B, C), dtype=np.float64)
    for core, res in enumerate(results):
        b = core // 4
        nn = np.asarray(res["nn_out"], dtype=np.float64)
        n[b] += nn[0]
        num[b] += nn[1]
    term = np.where(n > 0, num / (C * np.maximum(n, 1.0)), 0.0)
    return np.float32(term.sum())


def kernel(preds_S, preds_T, outputs_T):
    from concourse.bass_utils import run_bass_kernel_spmd

    nc = _get_program()
    in_maps = _make_in_maps(preds_S, preds_T, outputs_T)
    res = run_bass_kernel_spmd(nc, in_maps, core_ids=list(range(NCORES)))
    return np.asarray(_combine(res.results))


# revision 13
# speedup vs baseline: 1.7448x; 1.1323x over previous
"""Trainium2 Bass kernel for nn_BoundaryKDV4 (boundary-KL distillation loss).

Contract: kernel(**inputs) takes FULL inputs (preds_S, preds_T, outputs_T:
[2,14,96,96,96] f32), shards across 8 NeuronCores internally, and returns the
FULL output (scalar f32 loss), matching reference.py semantics.

Sharding: core = (b, hq) with b in {0,1}, hq in {0..3}; each core handles 24
H-slices of one batch. outputs_T shards carry a 1-slice halo on each side and
an extra "mask channel" (index 14) set to -2e38 on valid slices / +2e38 on
out-of-range halo slices, so the argmax one-hot of padding slices is
identically zero with a core-uniform SPMD program. All inputs are cast to
bf16 on the host (halves HBM traffic, doubles DVE throughput; the loss is an
average over ~1.8M voxels so the rounding noise washes out far below the
2e-2 gate). oT shards are pre-transposed to (slice, w, chan, d) so each DMA
is one contiguous run per partition.

Per-core pipeline (phase A and the oT/one-hot work interleaved so DMA, ACT,
DVE and PE all overlap):
  Phase A (KL): (c,g)-partition layout: p = c*9+g (126 partitions), free =
    u (V/9 per partition, chunked by 2048). expT/expS via ACT, q=expT*(sT-sS)
    on DVE; the three class sums go to the TENSOR engine as selector matmuls
    (sel[(c,g), g'] = [g==g']) accumulating [9, 512] row-blocks packed 12-up
    into [108, 512] PSUM banks. lnT/lnS are computed by ACT directly from
    PSUM; sumq is ACT-copied to SBUF; pk = sumq/sumT - ln sumT + ln sumS
    assembled on DVE in [108, 2048] tiles, bounced via DRAM into [96W, (h,d)].
  One-hot (interleaved with A): [96W, (s, c, d)] layout, 4 slices per load.
    Tree-max over 15 channels + broadcast is_ge -> one-hot (bf16), 26 slices.
  Phase B (boundary): H-box = 2 aligned bf16 shifted adds into a 128-row
    zero-padded tile (padding zeroed by DMA from a zeros blob, not memset).
    W-box+D-box = 3 PSUM-accumulated band matmuls on PE with a 128x128
    zero-padded tridiagonal weight (128 weights => FWL).
    ind = [0<box<27] via Square(box-13.5) < 169 (ACT + 4x tensor_scalar).
    n[k] = sum(ind), num[k] = sum(ind*pk) accumulated on PE with ones-column
    matmuls into two persistent PSUM banks, columns folded as (c, d%32) so
    the moving operand streams contiguously.
Host combines the 8 partial (n, num) pairs and applies the final
KLDivLoss(mean)-style normalization.
"""

import numpy as np

B, C, H, W, D = 2, 14, 96, 96, 96
CM = C + 1          # channels incl. mask
HQ = H // 4         # 24 h-slices per core
S = HQ + 2          # 26 oT slices incl. halo
WD = W * D          # 9216
V = HQ * WD         # 221184 voxels per core
NCORES = 8
BIG = 2.0e38        # +/- mask channel values (finite, beats any data)
WP = 128            # padded partition count for the band matmuls

# phase A (c,g) chunking: partition p = c*9+g, free u = V/9 = 24576
G = 9
U = V // G          # 24576 per partition
CG = C * G          # 126 partitions
UJ = 2048           # u-chunk per load step
NCB = U // UJ       # 12 load steps
NB = 4              # psum banks per summed tensor (12 row-blocks each)
JB = 12             # row-blocks (of 9 rows) per bank
ROWS = G * JB       # 108 rows per bank

_CACHE = {}


def _build_program():
    import concourse.bacc as bacc
    import concourse.mybir as mybir
    from concourse.mybir import AluOpType as alu
    from concourse.mybir import ActivationFunctionType as actf
    from concourse.tile import TileContext
    from contextlib import ExitStack
    import ml_dtypes

    f32 = mybir.dt.float32
    bf16 = mybir.dt.bfloat16
    bfnp = ml_dtypes.bfloat16

    nc = bacc.Bacc("TRN2", target_bir_lowering=False)

    # oT shard pre-transposed by host to (s, w, c, d); everything bf16
    ot = nc.dram_tensor("ot15", [S, W, CM, D], bf16, kind="ExternalInput")
    ps = nc.dram_tensor("ps", [C, V], bf16, kind="ExternalInput")
    pt = nc.dram_tensor("pt", [C, V], bf16, kind="ExternalInput")
    nn_out = nc.dram_tensor("nn_out", [2, C], f32, kind="ExternalOutput")

    # 128x128 zero-padded tridiagonal band: 128 weight columns enable FWL
    band_np = np.zeros((WP, WP), dtype=bfnp)
    for i in range(W):
        for j in range(max(0, i - 1), min(W, i + 2)):
            band_np[i, j] = 1.0
    band_h = nc.inline_tensor(band_np, name="bandw")
    ones_h = nc.inline_tensor(np.ones((W, 1), dtype=bfnp), name="onesw")
    # class-sum selectors, one per row-block j: sel_j[(c*9+g), j*9+g] = 1
    # (matmul out base partition must be 0/32/64, so each matmul writes the
    # full 108 rows and the 12 block-matmuls accumulate into one PSUM bank)
    sel_np = np.zeros((JB, CG, ROWS), dtype=bfnp)
    for j in range(JB):
        for c in range(C):
            for g in range(G):
                sel_np[j, c * G + g, j * G + g] = 1.0
    sel_h = nc.inline_tensor(sel_np, name="selcg")
    zer_h = nc.inline_tensor(
        np.zeros((WP - W, 4 * C * 100), dtype=bfnp), name="zeros32"
    )

    with TileContext(nc) as tc, ExitStack() as es:
        # ---------------- constants ----------------
        cpool = es.enter_context(tc.tile_pool(name="consts", bufs=1))
        band_t = cpool.tile([WP, WP], bf16, name="band_t")
        ones_t = cpool.tile([W, 1], bf16, name="ones_t")
        sel_t = cpool.tile([CG, JB, ROWS], bf16, name="sel_t")
        bias_t = cpool.tile([W, 1], f32, name="bias_t")
        nc.sync.dma_start(band_t[:], band_h[:])
        nc.sync.dma_start(ones_t[:], ones_h[:])
        nc.sync.dma_start(sel_t[:], sel_h[:].rearrange("j p r -> p j r"))
        nc.vector.memset(bias_t[:], -13.5)

        dram_pool = es.enter_context(tc.tile_pool(name="dramp", bufs=1, space="DRAM"))
        pk_dram = dram_pool.tile([V], bf16, name="pk_dram")

        # persistent one-hot for all 26 slices
        bpool = es.enter_context(tc.tile_pool(name="bconst", bufs=1))
        oh = bpool.tile([W, S, C, D], bf16, name="oh")
        pk_w = bpool.tile([W, HQ, D], bf16, name="pk_w")
        # two alternating 128-row H-box buffers; rows 96.. zeroed via DMA,
        # d-pad cols via small memsets (the band matmul streams these as rhs)
        hb_ts = [bpool.tile([WP, 4, C, 100], bf16, name=f"hb{i}") for i in range(2)]
        for hb in hb_ts:
            nc.sync.dma_start(
                hb[W:WP, :, :, :].rearrange("p a c d -> p (a c d)"), zer_h[:]
            )
            nc.vector.memset(hb[0:W, :, :, 0:2], 0.0)
            nc.vector.memset(hb[0:W, :, :, 98:100], 0.0)

        psum_acc = es.enter_context(tc.tile_pool(name="psacc", bufs=1, space="PSUM"))
        nacc = psum_acc.tile([1, 448], f32, name="nacc")
        numacc = psum_acc.tile([1, 448], f32, name="numacc")

        # class sums assembled in SBUF (bf16): [108, NB*512] in (j,g)(b,f) order
        apool = es.enter_context(tc.tile_pool(name="asums", bufs=1))
        lnT = apool.tile([ROWS, NB * 512], bf16, name="lnT")
        lnS = apool.tile([ROWS, NB * 512], bf16, name="lnS")
        sqS = apool.tile([ROWS, NB * 512], bf16, name="sqS")

        # phase A loads: p = (c,g), free = u-chunk
        ps_r = ps[:].rearrange("c (g cb f) -> cb (c g) f", g=G, cb=NCB, f=UJ)
        pt_r = pt[:].rearrange("c (g cb f) -> cb (c g) f", g=G, cb=NCB, f=UJ)

        mp = es.enter_context(tc.tile_pool(name="mwork", bufs=2))
        ap_es = ExitStack()
        alp = ap_es.enter_context(tc.tile_pool(name="aload", bufs=2))
        awp = ap_es.enter_context(tc.tile_pool(name="awork", bufs=2))
        otp = ap_es.enter_context(tc.tile_pool(name="otload", bufs=2))
        twp = ap_es.enter_context(tc.tile_pool(name="treework", bufs=2))
        csp = ap_es.enter_context(tc.tile_pool(name="csums", bufs=2, space="PSUM"))

        # per-tensor psum bank state: (tile, writes_done)
        bank_state = {}

        def emit_a_chunk(cb):
            sS_t = alp.tile([CG, UJ], bf16, name="sS_t", tag="sS")
            sT_t = alp.tile([CG, UJ], bf16, name="sT_t", tag="sT")
            nc.sync.dma_start(sS_t[:], ps_r[cb])
            nc.sync.dma_start(sT_t[:], pt_r[cb])

            expT = awp.tile([CG, UJ], bf16, name="expT", tag="expT")
            expS = awp.tile([CG, UJ], bf16, name="expS", tag="expS")
            q_t = awp.tile([CG, UJ], bf16, name="q_t", tag="q")
            nc.scalar.activation(expT[:], sT_t[:], actf.Exp)
            nc.scalar.activation(expS[:], sS_t[:], actf.Exp)
            # q = expT * (sT - sS), with the difference written in place
            nc.vector.tensor_tensor(q_t[:], sT_t[:], sS_t[:], alu.subtract)
            nc.vector.tensor_tensor(q_t[:], expT[:], q_t[:], alu.mult)

            # class sums on PE: one [9,512] row-block per 512-wide sub-chunk
            for ti, (src, dst) in enumerate(
                ((expT, lnT), (expS, lnS), (q_t, sqS))
            ):
                for qq in range(UJ // 512):
                    cb512 = cb * (UJ // 512) + qq
                    b, j = divmod(cb512, JB)
                    key = (ti, b)
                    if key not in bank_state:
                        bank_state[key] = [
                            csp.tile([ROWS, 512], f32, name=f"cs{ti}", tag=f"cs{ti}"),
                            0,
                        ]
                    bank, nw = bank_state[key]
                    nc.tensor.matmul(
                        bank[:],
                        sel_t[:, j, :],
                        src[:, 512 * qq : 512 * (qq + 1)],
                        start=(j == 0),
                        stop=(j == JB - 1),
                        skip_group_check=True,
                    )
                    bank_state[key][1] = nw + 1
                    if bank_state[key][1] == JB:
                        # bank complete: drain via ACT straight out of PSUM
                        out = dst[:, 512 * b : 512 * (b + 1)]
                        if ti == 2:
                            nc.scalar.activation(out, bank[:], actf.Copy)
                        else:
                            nc.scalar.activation(out, bank[:], actf.Ln)

        def emit_tree_cmp(s0, ns):
            """load oT slices [s0, s0+ns), tree-max over 15 ch, cmp -> oh"""
            oT_t = otp.tile([W, CM, 4, D], bf16, name="oT_t", tag="oT")
            for i in range(ns):
                nc.sync.dma_start(oT_t[:, :, i, :], ot[s0 + i])
            m_t = mp.tile([W, 4, D], bf16, name="m_t", tag="m")
            w7 = twp.tile([W, 7, 4, D], bf16, name="mw7", tag="mw7")
            w3 = twp.tile([W, 3, 4, D], bf16, name="mw3", tag="mw3")
            x = oT_t
            nc.vector.tensor_tensor(
                w7[:, :, 0:ns, :], x[:, 0:7, 0:ns, :], x[:, 7:14, 0:ns, :], alu.max
            )
            nc.vector.tensor_tensor(
                w3[:, :, 0:ns, :], w7[:, 0:3, 0:ns, :], w7[:, 3:6, 0:ns, :], alu.max
            )
            nc.vector.tensor_tensor(
                m_t[:, 0:ns, :], w3[:, 0, 0:ns, :], w3[:, 1, 0:ns, :], alu.max
            )
            nc.vector.tensor_tensor(
                m_t[:, 0:ns, :], m_t[:, 0:ns, :], w3[:, 2, 0:ns, :], alu.max
            )
            nc.vector.tensor_tensor(
                m_t[:, 0:ns, :], m_t[:, 0:ns, :], w7[:, 6, 0:ns, :], alu.max
            )
            nc.vector.tensor_tensor(
                m_t[:, 0:ns, :], m_t[:, 0:ns, :], x[:, 14, 0:ns, :], alu.max
            )
            # one-hot: broadcast compare over the class dim
            nc.vector.tensor_tensor(
                oh[:, s0 : s0 + ns, :, :],
                oT_t[:, 0:14, 0:ns, :].rearrange("w c s d -> w s c d"),
                m_t[:, 0:ns, None, :].broadcast_to([W, ns, C, D]),
                alu.is_ge,
            )

        # ---- interleaved emission: A-chunks + oT loads/one-hot ----
        # 7 tree_cmp loads of 4 slices (last 2) cover s in [0, 26)
        tc_sched = {0: [0], 1: [1], 2: [2], 4: [3], 6: [4], 8: [5], 10: [6]}
        for cb in range(NCB):
            emit_a_chunk(cb)
            for ld in tc_sched.get(cb, []):
                emit_tree_cmp(4 * ld, min(4, S - 4 * ld))

        # ---- voxel stage: pk = sumq/sumT - ln sumT + ln sumS ----
        with tc.tile_pool(name="avox", bufs=1) as avp:
            inv = avp.tile([ROWS, NB * 512], bf16, name="inv")
            pk_c = avp.tile([ROWS, NB * 512], bf16, name="pk_c")
            # inv = exp(-lnT) = 1/sumT
            nc.scalar.activation(inv[:], lnT[:], actf.Exp, scale=-1.0)
            nc.vector.tensor_tensor(pk_c[:], sqS[:], inv[:], alu.mult)
            nc.vector.tensor_tensor(pk_c[:], pk_c[:], lnT[:], alu.subtract)
            nc.vector.tensor_tensor(pk_c[:], pk_c[:], lnS[:], alu.add)
            # v = g*24576 + b*(12*512) + j*512 + f  (rows are (j,g), cols (b,f))
            nc.sync.dma_start(
                pk_dram[:].rearrange(
                    "(g b j f) -> j g b f", g=G, b=NB, j=JB, f=512
                ),
                pk_c[:],
            )
        ap_es.close()  # release phase-A pools before phase B allocates

        # pk in [96W, (h, d)] layout
        nc.sync.dma_start(
            pk_w[:], pk_dram[:].rearrange("(h w d) -> w h d", h=HQ, w=W, d=D)
        )

        # ---------------- phase B: boundary + reductions ----------------
        with tc.tile_pool(name="gwork", bufs=2) as gp, \
             tc.tile_pool(name="npwork", bufs=2) as npp, \
             tc.tile_pool(name="boxps", bufs=6, space="PSUM") as bxp:

            # c-chunks for the band matmuls (PSUM bank <= 512 f32 cols)
            CCH = [(0, 5), (5, 10), (10, 14)]
            first_cs = [True, True]  # per type (n, num): first colsum matmul?

            def emit_group(g0, ng, last_group):
                """boundary + accumulation for own slices g in [g0, g0+ng)"""
                # hb[j] = oh[g0+j] + oh[g0+j+1] + oh[g0+j+2] (d padded to 100)
                hb_t = hb_ts[(g0 // 4) % 2]
                nc.vector.tensor_tensor(
                    hb_t[0:W, 0:ng, :, 2 : 2 + D],
                    oh[:, g0 : g0 + ng, :, :],
                    oh[:, g0 + 2 : g0 + 2 + ng, :, :],
                    alu.add,
                )
                nc.vector.tensor_tensor(
                    hb_t[0:W, 0:ng, :, 2 : 2 + D],
                    hb_t[0:W, 0:ng, :, 2 : 2 + D],
                    oh[:, g0 + 1 : g0 + 1 + ng, :, :],
                    alu.add,
                )

                u_t = gp.tile([W, 4, C, D], bf16, name="u_t", tag="u")
                ind_t = gp.tile([W, 4, C, D], bf16, name="ind_t", tag="ind")
                np_t = npp.tile([W, 4, C, D], bf16, name="np_t", tag="np")

                for gi in range(ng):
                    for (c0, c1) in CCH:
                        cl = c1 - c0
                        box_ps = bxp.tile([WP, 5 * D], f32, name="box_ps", tag="box")
                        box_v = box_ps[:].rearrange("w (c d) -> w c d", c=5)
                        for dd in range(3):
                            nc.tensor.matmul(
                                box_v[:, 0:cl, :],
                                band_t[:],
                                hb_t[:, gi, c0:c1, 1 + dd : 1 + dd + D],
                                start=(dd == 0),
                                stop=(dd == 2),
                            )
                        # u = (box - 13.5)^2 ; boundary iff u < 169 (0<box<27)
                        nc.scalar.activation(
                            u_t[:, gi, c0:c1, :],
                            box_v[0:W, 0:cl, :],
                            actf.Square,
                            bias=bias_t[:],
                        )
                nc.vector.tensor_scalar(
                    ind_t[:, 0:ng, :, :], u_t[:, 0:ng, :, :], 169.0, None, alu.is_lt
                )
                nc.vector.tensor_tensor(
                    np_t[:, 0:ng, :, :],
                    ind_t[:, 0:ng, :, :],
                    pk_w[:, g0 : g0 + ng, None, :].broadcast_to([W, ng, C, D]),
                    alu.mult,
                )
                # colsum matmuls: accumulate per (type, g, d-chunk) into psum,
                # cols laid out (c, d % 32) so the rhs streams contiguously
                for gi in range(ng):
                    for ti, (src, accp) in enumerate(((ind_t, nacc), (np_t, numacc))):
                        for dc in range(3):
                            rhs = src[:, gi, :, 32 * dc : 32 * (dc + 1)]
                            is_first = first_cs[ti]
                            first_cs[ti] = False
                            is_last = last_group and gi == ng - 1 and dc == 2
                            nc.tensor.matmul(
                                accp[:].rearrange("p (c d) -> p c d", c=C),
                                ones_t[:],
                                rhs,
                                start=is_first,
                                stop=is_last,
                                skip_group_check=True,
                            )

            for k in range(6):
                emit_group(4 * k, 4, k == 5)

            # final: reduce (d%32) out of the accumulators, write [2, C]
            res_t = mp.tile([1, 2, C], f32, name="res_t", tag="res")
            nc.vector.tensor_reduce(
                res_t[:, 0, :],
                nacc[:].rearrange("p (c d) -> p c d", c=C),
                mybir.AxisListType.X,
                alu.add,
            )
            nc.vector.tensor_reduce(
                res_t[:, 1, :],
                numacc[:].rearrange("p (c d) -> p c d", c=C),
                mybir.AxisListType.X,
                alu.add,
            )
            nc.sync.dma_start(
                nn_out[:].rearrange("a c -> (a c)")[None, :],
                res_t[:].rearrange("p a c -> p (a c)"),
            )

    nc.compile()
    return nc


def _get_program():
    if "nc" not in _CACHE:
        _CACHE["nc"] = _build_program()
    return _CACHE["nc"]


def _make_in_maps(preds_S, preds_T, outputs_T):
    import ml_dtypes

    bfnp = ml_dtypes.bfloat16
    preds_S = np.asarray(preds_S)
    preds_T = np.asarray(preds_T)
    outputs_T = np.asarray(outputs_T)
    in_maps = []
    for core in range(NCORES):
        b, hq = divmod(core, 4)
        h0 = HQ * hq
        ot15 = np.empty((CM, S, W, D), dtype=np.float32)
        lo, hi = h0 - 1, h0 + HQ + 1
        slo, shi = max(0, lo), min(H, hi)
        ot15[:C, slo - lo : shi - lo] = outputs_T[b, :, slo:shi]
        ot15[C, :] = -BIG
        if lo < 0:
            ot15[:C, 0] = 0.0
            ot15[C, 0] = BIG
        if hi > H:
            ot15[:C, S - 1] = 0.0
            ot15[C, S - 1] = BIG
        in_maps.append(
            {
                # (c, s, w, d) -> (s, w, c, d): one contiguous run/partition
                "ot15": np.ascontiguousarray(
                    ot15.transpose(1, 2, 0, 3)
                ).astype(bfnp),
                "ps": np.ascontiguousarray(
                    preds_S[b, :, h0 : h0 + HQ].reshape(C, V)
                ).astype(bfnp),
                "pt": np.ascontiguousarray(
                    preds_T[b, :, h0 : h0 + HQ].reshape(C, V)
                ).astype(bfnp),
            }
        )
    return in_maps


def _combine(results):
    n = np.zeros((B, C), dtype=np.float64)
    num = np.zeros((B, C), dtype=np.float64)
    for core, res in enumerate(results):
        b = core // 4
        nn = np.asarray(res["nn_out"], dtype=np.float64)
        n[b] += nn[0]
        num[b] += nn[1]
    term = np.where(n > 0, num / (C * np.maximum(n, 1.0)), 0.0)
    return np.float32(term.sum())


def kernel(preds_S, preds_T, outputs_T):
    from concourse.bass_utils import run_bass_kernel_spmd

    nc = _get_program()
    in_maps = _make_in_maps(preds_S, preds_T, outputs_T)
    res = run_bass_kernel_spmd(nc, in_maps, core_ids=list(range(NCORES)))
    return np.asarray(_combine(res.results))
